# revision 47
# baseline (speedup 1.0000x reference)
"""Trainium2 Bass kernel for ViTDet-style attention with decomposed
relative-position bias.

Problem shapes (hardcoded):
  x: (4, 32, 32, 768) f32, Wqkv: (768, 2304), Wproj: (768, 768),
  bproj: (768,), rel_pos_h/w: (63, 64).
  12 heads, head_dim 64, S = 32*32 = 1024.

Sharding: 48 (batch, head) pairs -> 6 heads per core, all of one batch per
core-pair. Each core computes its heads' attention and a partial output
projection (its heads' channel rows of Wproj); the host sums the two
partials per batch and adds bproj.

Device algorithm per core (bf16 matmuls, fp32 PSUM accumulation):
  - qkT = Wqk^T @ x^T  (x^T supplied pre-transposed by host; k pre-scaled)
  - v   = x @ Wv       (natural layout); v_sb per-head segment is
    [v_i (64 cols) | ones (64 cols)] so the av matmul also produces the
    softmax denominator replicated on PSUM partitions 64-127.
  - rel-pos bias computed DIRECTLY in band form (no intermediate table
    product): BhT[r, (h,w)] = sum_c rhT[c, h+r] qT[c, (h,w)] via windowed
    stationaries; two h-values packed per 64x64 matmul (diagonal blocks
    used, off-diagonal garbage ignored).
  - scoresT (k x q) = kaugT^T @ qaugT in ONE K=128 matmul per tile:
    aug rows 0-63 = kT / qT, 64-95 = one-hot(h) / BhT, 96-127 = one-hot(w)/BwT
    => rel-pos bias folded into the QK matmul for free.
  - eT = exp(scoresT) on ScalarE (no max subtraction; scores are O(1)).
  - avT (128 x q): rows 0-63 = out accum, rows 64-127 = denominator.
  - normalize: DVE reciprocal of av[64:128] + DVE multiply. No DMA bounce.
  - partial = out_heads @ Wproj_shard  (natural layout, DMA PSUM->DRAM).
"""

import numpy as np

import concourse.bass as bass
import concourse.bacc as bacc
import concourse.mybir as mybir
import concourse.tile as tile
from concourse.bass_utils import run_bass_kernel_spmd

F32 = mybir.dt.float32
BF16 = mybir.dt.bfloat16

NH = 12          # total heads
C = 768
HD = 64
H = W = 32
S = H * W        # 1024
B = 4
NCORES = 8
HPC = NH * B // NCORES   # heads per core = 6
NCH = 6                  # C // 128 input-channel chunks
NKB = S // 128           # 8 k blocks
NQB = S // 128           # 8 q blocks
NHALF = 512              # matmul moving-dim half


def build_program():
    nc = bacc.Bacc("TRN2", target_bir_lowering=False, debug=False)

    xT = nc.declare_dram_parameter("xT", [C, S], BF16, isOutput=False)
    wqk = nc.declare_dram_parameter("wqk", [C, 2 * HPC * HD], BF16, isOutput=False)
    wv = nc.declare_dram_parameter("wv", [C, HPC * HD], BF16, isOutput=False)
    wproj = nc.declare_dram_parameter("wproj", [HPC * HD, C], BF16, isOutput=False)
    # raw h-table (transposed); w-table windowed: win[:, 64p+32j+r] = T[:, 2p+j+r]
    rh_tbl = nc.declare_dram_parameter("rh_tbl", [HD, 2 * H - 1], BF16,
                                       isOutput=False)
    rw_win = nc.declare_dram_parameter("rw_win", [HD, S], BF16, isOutput=False)
    onehot = nc.declare_dram_parameter("onehot", [64, S], BF16, isOutput=False)
    out = nc.declare_dram_parameter("out", [S, C], F32, isOutput=True)
    # DRAM bounce for the h-axis band gather (I/O tensors; internal DRAM
    # scratch is paged and slower for strided DMAs)
    ph_dram = nc.declare_dram_parameter("ph_dram", [(2 * H - 1) * S], BF16,
                                        isOutput=True)

    with tile.TileContext(nc) as tc:
        with (
            tc.tile_pool(name="persist", bufs=1) as persist,
            tc.tile_pool(name="ps_sc", bufs=2, space="PSUM") as ps_sc,
            tc.tile_pool(name="ps_aux", bufs=2, space="PSUM") as ps_aux,
            tc.tile_pool(name="et", bufs=4) as et_pool,
            tc.tile_pool(name="small", bufs=2) as small,
        ):
            # ---- persistent SBUF loads (interleaved so ci=0 compute can
            # start while later chunks stream in) ----
            xT_sb, wqk_sb, wv_sb = [], [], []
            for ci in range(NCH):
                t = persist.tile([128, S], BF16, tag=f"xT{ci}", name=f"xT{ci}")
                nc.sync.dma_start(t[:], xT[128 * ci:128 * (ci + 1), :])
                xT_sb.append(t)
                t = persist.tile([128, 2 * HPC * HD], BF16, tag=f"wqk{ci}",
                                 name=f"wqk{ci}")
                nc.sync.dma_start(t[:], wqk[128 * ci:128 * (ci + 1), :])
                wqk_sb.append(t)
                t = persist.tile([128, HPC * HD], BF16, tag=f"wv{ci}",
                                 name=f"wv{ci}")
                nc.sync.dma_start(t[:], wv[128 * ci:128 * (ci + 1), :])
                wv_sb.append(t)
            wproj_sb = []
            for ci in range(HPC * HD // 128):
                t = persist.tile([128, C], BF16, tag=f"wproj{ci}", name=f"wproj{ci}")
                nc.sync.dma_start(t[:], wproj[128 * ci:128 * (ci + 1), :])
                wproj_sb.append(t)
            rh_sb = persist.tile([HD, 2 * H - 1], BF16, tag="rh", name="rh_sb")
            nc.sync.dma_start(rh_sb[:], rh_tbl[:, :])
            rw_sb = persist.tile([HD, S], BF16, tag="rw", name="rw_sb")
            nc.sync.dma_start(rw_sb[:], rw_win[:, :])

            # ---- augmented k/q tiles (128, S) per head; one-hot rows DMAed
            # straight from DRAM into kaug rows 64-127 ----
            kaug = [persist.tile([128, S], BF16, tag=f"kaug{i}", name=f"kaug{i}")
                    for i in range(HPC)]
            qaug = [persist.tile([128, S], BF16, tag=f"qaug{i}", name=f"qaug{i}")
                    for i in range(HPC)]
            for i in range(HPC):
                nc.sync.dma_start(kaug[i][64:128, :], onehot[:, :])

            # ---- v tiles: per-head segment [ones (64) | v_i (64)] ----
            # (ones first so the av denominator lands on PSUM partitions 0-63:
            # reciprocal_approx_fast misreads PSUM at base_partition >= 64)
            v_sb = [persist.tile([128, HPC * 2 * HD], BF16, tag=f"v{sb}",
                                 name=f"v{sb}")
                    for sb in range(NKB)]
            for sb in range(NKB):
                ones_dst = bass.AP(v_sb[sb].tensor, v_sb[sb][:].offset,
                                   [v_sb[sb][:].ap[0], [2 * HD, HPC], [1, HD]])
                nc.gpsimd.memset(ones_dst, 1.0)

            # warm the exp activation table during the DMA lead-in
            warm = small.tile([1, 2], F32, tag="warm", name="warm")
            nc.gpsimd.memset(warm[:], 0.0)
            nc.scalar.activation(warm[:], warm[:],
                                 mybir.ActivationFunctionType.Exp)

            # ---- v projection (natural) ----
            for sb in range(NKB):
                vp = ps_aux.tile([128, S], F32, tag="aux", name="vp")
                for ci in range(NCH):
                    nc.tensor.matmul(
                        vp[:, 0:HPC * HD],
                        xT_sb[ci][:, 128 * sb:128 * (sb + 1)],
                        wv_sb[ci][:],
                        start=(ci == 0), stop=(ci == NCH - 1))
                src = bass.AP(vp.tensor, vp[:].offset,
                              [vp[:].ap[0], [HD, HPC], [1, HD]])
                dst = bass.AP(v_sb[sb].tensor, v_sb[sb][:].offset + HD,
                              [v_sb[sb][:].ap[0], [2 * HD, HPC], [1, HD]])
                nc.vector.tensor_copy(dst, src)

            # ---- qk projection (transposed layout) ----
            # qkT octile t covers oc rows [128t, 128t+128): t<3 -> q, t>=3 -> k
            def qk_octile(t):
                qp = ps_aux.tile([128, S], F32, tag="aux", name="qp")
                for ci in range(NCH):
                    for nh in range(S // NHALF):
                        nc.tensor.matmul(
                            qp[:, NHALF * nh:NHALF * (nh + 1)],
                            wqk_sb[ci][:, 128 * t:128 * (t + 1)],
                            xT_sb[ci][:, NHALF * nh:NHALF * (nh + 1)],
                            start=(ci == 0), stop=(ci == NCH - 1))
                for sub in range(2):
                    head = (t % 3) * 2 + sub
                    dst = (qaug if t < 3 else kaug)[head]
                    nc.scalar.copy(dst[0:64, :], qp[64 * sub:64 * sub + 64, :])

            # ---- per head: direct banded rel-pos bias into qaug rows 64-127
            # BhT[r, (h,w)] = sum_c rhT[c, h+r] qT[c, (h,w)]  (r, h in [0,32))
            # Two h-values per matmul: stationary (64, 64) = two overlapping
            # 32-col windows of the table; useful output = diagonal blocks.
            def band_extract(i):
                # heads 0-1 run in the lead-in where ScalarE is idle; the
                # rest overlap the exp-bound phase where VectorE has slack
                cp = nc.scalar.copy if i < 2 else nc.vector.tensor_copy
                # h-axis: PhT[j, q] = sum_c rhT[c, j] qT[c, q], then the band
                # shift qaug[64+r, (h,w)] = PhT[h+r, (h,w)] via one strided
                # SBUF->SBUF DMA (engines stay free).
                ph = ps_aux.tile([2 * H - 1, S], F32, tag="aux", name="ph")
                for nh in range(S // NHALF):
                    sl = slice(NHALF * nh, NHALF * (nh + 1))
                    nc.tensor.matmul(ph[:, sl], rh_sb[:],
                                     qaug[i][0:64, sl], start=True, stop=True)
                ph_sb = small.tile([2 * H - 1, S], BF16, tag="ph_sb",
                                   name="ph_sb")
                cp(ph_sb[:], ph[:])
                nc.sync.dma_start(bass.AP(ph_dram, 0, [[S, 2 * H - 1], [1, S]]),
                                  ph_sb[:])
                nc.sync.dma_start(
                    bass.AP(qaug[i].tensor, qaug[i][:].offset + 64 * S,
                            [[S, 32], [W, H], [1, W]]),
                    bass.AP(ph_dram, 0, [[S, 32], [S + W, H], [1, W]]))
                bw = ps_aux.tile([64, S], F32, tag="aux", name="bw")
                pitch = bw[:].ap[0][0]
                for p in range(16):
                    rw = bass.AP(qaug[i].tensor, qaug[i][:].offset + 2 * p,
                                 [[S, 64], [1, 2], [W, H]])
                    nc.tensor.matmul(bw[:, 64 * p:64 * (p + 1)],
                                     rw_sb[:, 64 * p:64 * (p + 1)],
                                     rw, start=True, stop=True)
                # w-axis, j=0: w=2p / j=1: w=2p+1; dst col = 32h + w
                cp(bass.AP(qaug[i].tensor, qaug[i][:].offset + 96 * S,
                           [[S, 32], [2, 16], [W, H]]),
                   bass.AP(bw.tensor, bw[:].offset,
                           [[pitch, 32], [64, 16], [1, 32]]))
                cp(bass.AP(qaug[i].tensor, qaug[i][:].offset + 96 * S + 1,
                           [[S, 32], [2, 16], [W, H]]),
                   bass.AP(bw.tensor, bw[:].offset + 32 * pitch + 32,
                           [[pitch, 32], [64, 16], [1, 32]]))

            # ---- attention per head; band extraction two heads ahead ----
            out_headsT = [persist.tile([128, S], BF16, tag=f"ohT{c}",
                                       name=f"ohT{c}")
                          for c in range(HPC * HD // 128)]

            for t in [0, 3, 1, 4, 2, 5]:
                qk_octile(t)
            band_extract(0)
            band_extract(1)
            for i in range(HPC):
                if i + 2 < HPC:
                    band_extract(i + 2)
                av = ps_aux.tile([128, S], F32, tag="aux", name="av")
                for kb in range(NKB):
                    sc = ps_sc.tile([128, S], F32, tag="sc", name="sc")
                    for nh in range(S // NHALF):
                        sl = slice(NHALF * nh, NHALF * (nh + 1))
                        nc.tensor.matmul(
                            sc[:, sl],
                            kaug[i][:, 128 * kb:128 * (kb + 1)],
                            qaug[i][:, sl], start=True, stop=True)
                    e = et_pool.tile([128, S], BF16, tag="et", name="et")
                    nc.scalar.activation(e[:], sc[:],
                                         mybir.ActivationFunctionType.Exp)
                    for nh in range(S // NHALF):
                        sl = slice(NHALF * nh, NHALF * (nh + 1))
                        nc.tensor.matmul(
                            av[:, sl],
                            v_sb[kb][:, 2 * HD * i:2 * HD * (i + 1)],
                            e[:, sl],
                            start=(kb == 0), stop=(kb == NKB - 1))
                rb = small.tile([64, S], F32, tag="rb", name="rb")
                nc.vector.reciprocal_approx_fast(rb[:], av[0:64, :])
                chunk, row = i // 2, (i % 2) * 64
                nc.vector.tensor_tensor(
                    out_headsT[chunk][row:row + 64, :], av[64:128, :], rb[:],
                    op=mybir.AluOpType.mult)

            # ---- output projection (partial); pp tiles alternate across
            # both PSUM pools so the tail drains through 4 slots ----
            for qb in range(NQB):
                pool, tag = (ps_sc, "sc") if qb % 2 else (ps_aux, "aux")
                pp = pool.tile([128, S], F32, tag=tag, name="pp")
                for ci in range(HPC * HD // 128):
                    nc.tensor.matmul(
                        pp[:, 0:NHALF],
                        out_headsT[ci][:, 128 * qb:128 * (qb + 1)],
                        wproj_sb[ci][:, 0:NHALF],
                        start=(ci == 0), stop=(ci == 2))
                    nc.tensor.matmul(
                        pp[:, NHALF:C],
                        out_headsT[ci][:, 128 * qb:128 * (qb + 1)],
                        wproj_sb[ci][:, NHALF:C],
                        start=(ci == 0), stop=(ci == 2))
                pp_sb = small.tile([128, C], F32, tag="pp_sb", name="pp_sb")
                (nc.scalar.copy if qb % 2 else nc.vector.tensor_copy)(
                    pp_sb[:], pp[:, 0:C])
                nc.sync.dma_start(out[128 * qb:128 * (qb + 1), :], pp_sb[:])

    nc.compile()
    return nc


def shard_inputs(x, Wqkv, Wproj, rel_pos_h, rel_pos_w):
    """Build the 8 per-core input maps."""
    import ml_dtypes
    bf16 = ml_dtypes.bfloat16
    scale = HD ** (-0.5)
    x = np.asarray(x, dtype=np.float32)
    Wqkv = np.asarray(Wqkv, dtype=np.float32)
    Wproj = np.asarray(Wproj, dtype=np.float32)
    rhT = np.ascontiguousarray(np.asarray(rel_pos_h, np.float32).T)
    rwT = np.ascontiguousarray(np.asarray(rel_pos_w, np.float32).T)

    def windowed(T):
        win = np.zeros((HD, S), np.float32)
        for p in range(16):
            for j in range(2):
                win[:, 64 * p + 32 * j:64 * p + 32 * j + 32] = \
                    T[:, 2 * p + j:2 * p + j + 32]
        return win.astype(bf16)

    rh_tbl = rhT.astype(bf16)
    rw_win = windowed(rwT)
    oh = np.zeros((64, S), np.float32)
    for khp in range(H):
        oh[khp, (31 - khp) * W:(31 - khp) * W + W] = 1.0
    for kwp in range(W):
        oh[32 + kwp, 31 - kwp::W] = 1.0
    oh = oh.astype(bf16)
    in_maps = []
    for core in range(NCORES):
        b = core // 2
        h0 = (core % 2) * HPC
        xb = x[b].reshape(S, C)
        xT = np.ascontiguousarray(xb.T).astype(bf16)
        wq = Wqkv[:, h0 * HD:(h0 + HPC) * HD]
        wk = Wqkv[:, C + h0 * HD:C + (h0 + HPC) * HD] * scale
        wqk = np.ascontiguousarray(np.concatenate([wq, wk], axis=1)).astype(bf16)
        wv = np.ascontiguousarray(
            Wqkv[:, 2 * C + h0 * HD:2 * C + (h0 + HPC) * HD]).astype(bf16)
        wp = np.ascontiguousarray(Wproj[h0 * HD:(h0 + HPC) * HD, :]).astype(bf16)
        in_maps.append({"xT": xT, "wqk": wqk, "wv": wv, "wproj": wp,
                        "rh_tbl": rh_tbl, "rw_win": rw_win, "onehot": oh})
    return in_maps


_NC_CACHE = {}


def kernel(x, Wqkv, Wproj, bproj, rel_pos_h, rel_pos_w):
    if "nc" not in _NC_CACHE:
        _NC_CACHE["nc"] = build_program()
    nc = _NC_CACHE["nc"]
    in_maps = shard_inputs(x, Wqkv, Wproj, rel_pos_h, rel_pos_w)
    res = run_bass_kernel_spmd(nc, in_maps, list(range(NCORES)))
    bproj = np.asarray(bproj, dtype=np.float32)
    out = np.empty((B, H, W, C), dtype=np.float32)
    for b in range(B):
        acc = res.results[2 * b]["out"] + res.results[2 * b + 1]["out"] + bproj
        out[b] = acc.reshape(H, W, C)
    return out


# revision 48
# speedup vs baseline: 1.0131x; 1.0131x over previous
"""Trainium2 Bass kernel for ViTDet-style attention with decomposed
relative-position bias.

Problem shapes (hardcoded):
  x: (4, 32, 32, 768) f32, Wqkv: (768, 2304), Wproj: (768, 768),
  bproj: (768,), rel_pos_h/w: (63, 64).
  12 heads, head_dim 64, S = 32*32 = 1024.

Sharding: 48 (batch, head) pairs -> 6 heads per core, all of one batch per
core-pair. Each core computes its heads' attention and a partial output
projection (its heads' channel rows of Wproj); the host sums the two
partials per batch and adds bproj.

Device algorithm per core (bf16 matmuls, fp32 PSUM accumulation):
  - qkT = Wqk^T @ x^T  (x^T supplied pre-transposed by host; k pre-scaled)
  - v   = x @ Wv       (natural layout); v_sb per-head segment is
    [v_i (64 cols) | ones (64 cols)] so the av matmul also produces the
    softmax denominator replicated on PSUM partitions 64-127.
  - rel-pos bias computed DIRECTLY in band form (no intermediate table
    product): BhT[r, (h,w)] = sum_c rhT[c, h+r] qT[c, (h,w)] via windowed
    stationaries; two h-values packed per 64x64 matmul (diagonal blocks
    used, off-diagonal garbage ignored).
  - scoresT (k x q) = kaugT^T @ qaugT in ONE K=128 matmul per tile:
    aug rows 0-63 = kT / qT, 64-95 = one-hot(h) / BhT, 96-127 = one-hot(w)/BwT
    => rel-pos bias folded into the QK matmul for free.
  - eT = exp(scoresT) on ScalarE (no max subtraction; scores are O(1)).
  - avT (128 x q): rows 0-63 = out accum, rows 64-127 = denominator.
  - normalize: DVE reciprocal of av[64:128] + DVE multiply. No DMA bounce.
  - partial = out_heads @ Wproj_shard  (natural layout, DMA PSUM->DRAM).
"""

import numpy as np

import concourse.bass as bass
import concourse.bacc as bacc
import concourse.mybir as mybir
import concourse.tile as tile
from concourse.bass_utils import run_bass_kernel_spmd

F32 = mybir.dt.float32
BF16 = mybir.dt.bfloat16

NH = 12          # total heads
C = 768
HD = 64
H = W = 32
S = H * W        # 1024
B = 4
NCORES = 8
HPC = NH * B // NCORES   # heads per core = 6
NCH = 6                  # C // 128 input-channel chunks
NKB = S // 128           # 8 k blocks
NQB = S // 128           # 8 q blocks
NHALF = 512              # matmul moving-dim half


def build_program():
    nc = bacc.Bacc("TRN2", target_bir_lowering=False, debug=False)

    xT = nc.declare_dram_parameter("xT", [C, S], BF16, isOutput=False)
    wqk = nc.declare_dram_parameter("wqk", [C, 2 * HPC * HD], BF16, isOutput=False)
    wv = nc.declare_dram_parameter("wv", [C, HPC * HD], BF16, isOutput=False)
    wproj = nc.declare_dram_parameter("wproj", [HPC * HD, C], BF16, isOutput=False)
    # raw h-table (transposed); w-table windowed: win[:, 64p+32j+r] = T[:, 2p+j+r]
    rh_tbl = nc.declare_dram_parameter("rh_tbl", [HD, 2 * H - 1], BF16,
                                       isOutput=False)
    rw_win = nc.declare_dram_parameter("rw_win", [HD, S], BF16, isOutput=False)
    onehot = nc.declare_dram_parameter("onehot", [64, S], BF16, isOutput=False)
    out = nc.declare_dram_parameter("out", [S, C], F32, isOutput=True)
    # DRAM bounce for the h-axis band gather (I/O tensors; internal DRAM
    # scratch is paged and slower for strided DMAs)
    ph_dram = nc.declare_dram_parameter("ph_dram", [(2 * H - 1) * S], BF16,
                                        isOutput=True)

    with tile.TileContext(nc) as tc:
        with (
            tc.tile_pool(name="persist", bufs=1) as persist,
            tc.tile_pool(name="ps_sc", bufs=2, space="PSUM") as ps_sc,
            tc.tile_pool(name="ps_aux", bufs=2, space="PSUM") as ps_aux,
            tc.tile_pool(name="et", bufs=4) as et_pool,
            tc.tile_pool(name="small", bufs=2) as small,
        ):
            # ---- persistent SBUF loads (interleaved so ci=0 compute can
            # start while later chunks stream in) ----
            xT_sb, wqk_sb, wv_sb = [], [], []
            for ci in range(NCH):
                t = persist.tile([128, S], BF16, tag=f"xT{ci}", name=f"xT{ci}")
                nc.sync.dma_start(t[:], xT[128 * ci:128 * (ci + 1), :])
                xT_sb.append(t)
                t = persist.tile([128, 2 * HPC * HD], BF16, tag=f"wqk{ci}",
                                 name=f"wqk{ci}")
                nc.sync.dma_start(t[:], wqk[128 * ci:128 * (ci + 1), :])
                wqk_sb.append(t)
                t = persist.tile([128, HPC * HD], BF16, tag=f"wv{ci}",
                                 name=f"wv{ci}")
                nc.sync.dma_start(t[:], wv[128 * ci:128 * (ci + 1), :])
                wv_sb.append(t)
            wproj_sb = []
            for ci in range(HPC * HD // 128):
                t = persist.tile([128, C], BF16, tag=f"wproj{ci}", name=f"wproj{ci}")
                nc.sync.dma_start(t[:], wproj[128 * ci:128 * (ci + 1), :])
                wproj_sb.append(t)
            rh_sb = persist.tile([HD, 2 * H - 1], BF16, tag="rh", name="rh_sb")
            nc.sync.dma_start(rh_sb[:], rh_tbl[:, :])
            rw_sb = persist.tile([HD, S], BF16, tag="rw", name="rw_sb")
            nc.sync.dma_start(rw_sb[:], rw_win[:, :])

            # ---- augmented k/q tiles (128, S) per head; one-hot rows DMAed
            # straight from DRAM into kaug rows 64-127 ----
            kaug = [persist.tile([128, S], BF16, tag=f"kaug{i}", name=f"kaug{i}")
                    for i in range(HPC)]
            qaug = [persist.tile([128, S], BF16, tag=f"qaug{i}", name=f"qaug{i}")
                    for i in range(HPC)]
            for i in range(HPC):
                nc.sync.dma_start(kaug[i][64:128, :], onehot[:, :])

            # ---- v tiles: per-head segment [ones (64) | v_i (64)] ----
            # (ones first so the av denominator lands on PSUM partitions 0-63:
            # reciprocal_approx_fast misreads PSUM at base_partition >= 64)
            v_sb = [persist.tile([128, HPC * 2 * HD], BF16, tag=f"v{sb}",
                                 name=f"v{sb}")
                    for sb in range(NKB)]
            for sb in range(NKB):
                ones_dst = bass.AP(v_sb[sb].tensor, v_sb[sb][:].offset,
                                   [v_sb[sb][:].ap[0], [2 * HD, HPC], [1, HD]])
                nc.gpsimd.memset(ones_dst, 1.0)

            # warm the exp activation table during the DMA lead-in
            warm = small.tile([1, 2], F32, tag="warm", name="warm")
            nc.gpsimd.memset(warm[:], 0.0)
            nc.scalar.activation(warm[:], warm[:],
                                 mybir.ActivationFunctionType.Exp)

            # ---- v projection (natural) ----
            for sb in range(NKB):
                vp = ps_aux.tile([128, S], F32, tag="aux", name="vp")
                for ci in range(NCH):
                    nc.tensor.matmul(
                        vp[:, 0:HPC * HD],
                        xT_sb[ci][:, 128 * sb:128 * (sb + 1)],
                        wv_sb[ci][:],
                        start=(ci == 0), stop=(ci == NCH - 1))
                src = bass.AP(vp.tensor, vp[:].offset,
                              [vp[:].ap[0], [HD, HPC], [1, HD]])
                dst = bass.AP(v_sb[sb].tensor, v_sb[sb][:].offset + HD,
                              [v_sb[sb][:].ap[0], [2 * HD, HPC], [1, HD]])
                nc.vector.tensor_copy(dst, src)

            # ---- qk projection (transposed layout) ----
            # qkT octile t covers oc rows [128t, 128t+128): t<3 -> q, t>=3 -> k
            def qk_octile(t):
                qp = ps_aux.tile([128, S], F32, tag="aux", name="qp")
                for ci in range(NCH):
                    for nh in range(S // NHALF):
                        nc.tensor.matmul(
                            qp[:, NHALF * nh:NHALF * (nh + 1)],
                            wqk_sb[ci][:, 128 * t:128 * (t + 1)],
                            xT_sb[ci][:, NHALF * nh:NHALF * (nh + 1)],
                            start=(ci == 0), stop=(ci == NCH - 1))
                for sub in range(2):
                    head = (t % 3) * 2 + sub
                    dst = (qaug if t < 3 else kaug)[head]
                    nc.scalar.copy(dst[0:64, :], qp[64 * sub:64 * sub + 64, :])

            # ---- per head: direct banded rel-pos bias into qaug rows 64-127
            # BhT[r, (h,w)] = sum_c rhT[c, h+r] qT[c, (h,w)]  (r, h in [0,32))
            # Two h-values per matmul: stationary (64, 64) = two overlapping
            # 32-col windows of the table; useful output = diagonal blocks.
            def band_extract(i):
                # heads 0-1 run in the lead-in where ScalarE is idle; the
                # rest overlap the exp-bound phase where VectorE has slack
                cp = nc.scalar.copy if i < 2 else nc.vector.tensor_copy
                # h-axis: PhT[j, q] = sum_c rhT[c, j] qT[c, q], then the band
                # shift qaug[64+r, (h,w)] = PhT[h+r, (h,w)] via one strided
                # SBUF->SBUF DMA (engines stay free).
                ph = ps_aux.tile([2 * H - 1, S], F32, tag="aux", name="ph")
                for nh in range(S // NHALF):
                    sl = slice(NHALF * nh, NHALF * (nh + 1))
                    nc.tensor.matmul(ph[:, sl], rh_sb[:],
                                     qaug[i][0:64, sl], start=True, stop=True)
                ph_sb = small.tile([2 * H - 1, S], BF16, tag="ph_sb",
                                   name="ph_sb")
                cp(ph_sb[:], ph[:])
                nc.sync.dma_start(bass.AP(ph_dram, 0, [[S, 2 * H - 1], [1, S]]),
                                  ph_sb[:])
                nc.sync.dma_start(
                    bass.AP(qaug[i].tensor, qaug[i][:].offset + 64 * S,
                            [[S, 32], [W, H], [1, W]]),
                    bass.AP(ph_dram, 0, [[S, 32], [S + W, H], [1, W]]))
                bw = ps_aux.tile([64, S], F32, tag="aux", name="bw")
                pitch = bw[:].ap[0][0]
                for p in range(16):
                    rw = bass.AP(qaug[i].tensor, qaug[i][:].offset + 2 * p,
                                 [[S, 64], [1, 2], [W, H]])
                    nc.tensor.matmul(bw[:, 64 * p:64 * (p + 1)],
                                     rw_sb[:, 64 * p:64 * (p + 1)],
                                     rw, start=True, stop=True)
                # w-axis, j=0: w=2p / j=1: w=2p+1; dst col = 32h + w
                cp(bass.AP(qaug[i].tensor, qaug[i][:].offset + 96 * S,
                           [[S, 32], [2, 16], [W, H]]),
                   bass.AP(bw.tensor, bw[:].offset,
                           [[pitch, 32], [64, 16], [1, 32]]))
                cp(bass.AP(qaug[i].tensor, qaug[i][:].offset + 96 * S + 1,
                           [[S, 32], [2, 16], [W, H]]),
                   bass.AP(bw.tensor, bw[:].offset + 32 * pitch + 32,
                           [[pitch, 32], [64, 16], [1, 32]]))

            # ---- attention per head; band extraction two heads ahead ----
            out_headsT = [persist.tile([128, S], BF16, tag=f"ohT{c}",
                                       name=f"ohT{c}")
                          for c in range(HPC * HD // 128)]

            # only head 0/1's octiles ahead of the loop; the rest are emitted
            # just before the first band that needs them, so their PE work
            # overlaps the exp-bound attention phase
            qk_octile(0)
            qk_octile(3)
            band_extract(0)
            band_extract(1)
            for i in range(HPC):
                if i + 2 < HPC:
                    if i % 2 == 0:
                        qk_octile((i + 2) // 2)
                        qk_octile((i + 2) // 2 + 3)
                    band_extract(i + 2)
                av = ps_aux.tile([128, S], F32, tag="aux", name="av")
                for kb in range(NKB):
                    sc = ps_sc.tile([128, S], F32, tag="sc", name="sc")
                    for nh in range(S // NHALF):
                        sl = slice(NHALF * nh, NHALF * (nh + 1))
                        nc.tensor.matmul(
                            sc[:, sl],
                            kaug[i][:, 128 * kb:128 * (kb + 1)],
                            qaug[i][:, sl], start=True, stop=True)
                    e = et_pool.tile([128, S], BF16, tag="et", name="et")
                    nc.scalar.activation(e[:], sc[:],
                                         mybir.ActivationFunctionType.Exp)
                    for nh in range(S // NHALF):
                        sl = slice(NHALF * nh, NHALF * (nh + 1))
                        nc.tensor.matmul(
                            av[:, sl],
                            v_sb[kb][:, 2 * HD * i:2 * HD * (i + 1)],
                            e[:, sl],
                            start=(kb == 0), stop=(kb == NKB - 1))
                rb = small.tile([64, S], F32, tag="rb", name="rb")
                nc.vector.reciprocal_approx_fast(rb[:], av[0:64, :])
                chunk, row = i // 2, (i % 2) * 64
                nc.vector.tensor_tensor(
                    out_headsT[chunk][row:row + 64, :], av[64:128, :], rb[:],
                    op=mybir.AluOpType.mult)

            # ---- output projection (partial); pp tiles alternate across
            # both PSUM pools so the tail drains through 4 slots ----
            for qb in range(NQB):
                pool, tag = (ps_sc, "sc") if qb % 2 else (ps_aux, "aux")
                pp = pool.tile([128, S], F32, tag=tag, name="pp")
                for ci in range(HPC * HD // 128):
                    nc.tensor.matmul(
                        pp[:, 0:NHALF],
                        out_headsT[ci][:, 128 * qb:128 * (qb + 1)],
                        wproj_sb[ci][:, 0:NHALF],
                        start=(ci == 0), stop=(ci == 2))
                    nc.tensor.matmul(
                        pp[:, NHALF:C],
                        out_headsT[ci][:, 128 * qb:128 * (qb + 1)],
                        wproj_sb[ci][:, NHALF:C],
                        start=(ci == 0), stop=(ci == 2))
                pp_sb = small.tile([128, C], F32, tag="pp_sb", name="pp_sb")
                (nc.scalar.copy if qb % 2 else nc.vector.tensor_copy)(
                    pp_sb[:], pp[:, 0:C])
                nc.sync.dma_start(out[128 * qb:128 * (qb + 1), :], pp_sb[:])

    nc.compile()
    return nc


def shard_inputs(x, Wqkv, Wproj, rel_pos_h, rel_pos_w):
    """Build the 8 per-core input maps."""
    import ml_dtypes
    bf16 = ml_dtypes.bfloat16
    scale = HD ** (-0.5)
    x = np.asarray(x, dtype=np.float32)
    Wqkv = np.asarray(Wqkv, dtype=np.float32)
    Wproj = np.asarray(Wproj, dtype=np.float32)
    rhT = np.ascontiguousarray(np.asarray(rel_pos_h, np.float32).T)
    rwT = np.ascontiguousarray(np.asarray(rel_pos_w, np.float32).T)

    def windowed(T):
        win = np.zeros((HD, S), np.float32)
        for p in range(16):
            for j in range(2):
                win[:, 64 * p + 32 * j:64 * p + 32 * j + 32] = \
                    T[:, 2 * p + j:2 * p + j + 32]
        return win.astype(bf16)

    rh_tbl = rhT.astype(bf16)
    rw_win = windowed(rwT)
    oh = np.zeros((64, S), np.float32)
    for khp in range(H):
        oh[khp, (31 - khp) * W:(31 - khp) * W + W] = 1.0
    for kwp in range(W):
        oh[32 + kwp, 31 - kwp::W] = 1.0
    oh = oh.astype(bf16)
    in_maps = []
    for core in range(NCORES):
        b = core // 2
        h0 = (core % 2) * HPC
        xb = x[b].reshape(S, C)
        xT = np.ascontiguousarray(xb.T).astype(bf16)
        wq = Wqkv[:, h0 * HD:(h0 + HPC) * HD]
        wk = Wqkv[:, C + h0 * HD:C + (h0 + HPC) * HD] * scale
        wqk = np.ascontiguousarray(np.concatenate([wq, wk], axis=1)).astype(bf16)
        wv = np.ascontiguousarray(
            Wqkv[:, 2 * C + h0 * HD:2 * C + (h0 + HPC) * HD]).astype(bf16)
        wp = np.ascontiguousarray(Wproj[h0 * HD:(h0 + HPC) * HD, :]).astype(bf16)
        in_maps.append({"xT": xT, "wqk": wqk, "wv": wv, "wproj": wp,
                        "rh_tbl": rh_tbl, "rw_win": rw_win, "onehot": oh})
    return in_maps


_NC_CACHE = {}


def kernel(x, Wqkv, Wproj, bproj, rel_pos_h, rel_pos_w):
    if "nc" not in _NC_CACHE:
        _NC_CACHE["nc"] = build_program()
    nc = _NC_CACHE["nc"]
    in_maps = shard_inputs(x, Wqkv, Wproj, rel_pos_h, rel_pos_w)
    res = run_bass_kernel_spmd(nc, in_maps, list(range(NCORES)))
    bproj = np.asarray(bproj, dtype=np.float32)
    out = np.empty((B, H, W, C), dtype=np.float32)
    for b in range(B):
        acc = res.results[2 * b]["out"] + res.results[2 * b + 1]["out"] + bproj
        out[b] = acc.reshape(H, W, C)
    return out
